# revision 50
# baseline (speedup 1.0000x reference)
"""CRFVGG_prune message-passing kernel for 8 TRN2 NeuronCores.

Structure: 3 node types with channel counts [228, 111, 51] hold [B,C,256,256]
feature maps. Two message-passing iterations of 1x1 convs (per-pixel matmuls)
between all ordered node pairs, each followed by prelu + residual + relu.

Sharding: pure data parallel over B*H rows (512 rows -> 64 rows/core).
Per-core layout is channels-on-partitions: x [390, 16384px] bf16 in four
chunks (h0a|h0b|h1|h2). All matmuls are bf16 (fp32 PSUM accum); every weight
tile is zero-padded to 128 columns so LdWeights takes the FWL fast path.
Elementwise in bf16, output written bf16 and upcast on host.
"""
import os
import sys

sys.path.insert(0, "/opt/trn_rl_repo")

import numpy as np
import ml_dtypes

import concourse.bass as bass
import concourse.tile as tile
from concourse import bacc, mybir
from concourse.bass_utils import run_bass_kernel_spmd

BF16 = ml_dtypes.bfloat16
BF = mybir.dt.bfloat16
F32 = mybir.dt.float32

B, H, W = 2, 256, 256
CHS = [228, 111, 51]
CTOT = sum(CHS)  # 390
NCORES = 8
ROWS_PER_CORE = (B * H) // NCORES  # 64
PX = ROWS_PER_CORE * W  # 16384 pixels per core
MACRO = 2048  # pixels per macro tile (one PSUM tile = 4 banks; 2 double-buffer)
NMACRO = PX // MACRO
NSUB = MACRO // 512  # matmul N-subtiles per macro

# chunk: (out_row_start in 390-layout, n_rows, x_row_start in padded 512-layout)
CHUNKS = {
    "0a": (0, 128, 0),
    "0b": (128, 100, 128),
    "1": (228, 111, 256),
    "2": (339, 51, 384),
}
# target chunk -> list of (source chunk, weight key, K-slice, M-slice)
# weight w_j_i: [cout_i, cin_j]; lhsT piece = w_j_i.T[kslice, mslice]
TARGETS = {
    "0a": [("1", "w_1_0", (0, 111), (0, 128)), ("2", "w_2_0", (0, 51), (0, 128))],
    "0b": [("1", "w_1_0", (0, 111), (128, 228)), ("2", "w_2_0", (0, 51), (128, 228))],
    "1": [
        ("0a", "w_0_1", (0, 128), (0, 111)),
        ("0b", "w_0_1", (128, 228), (0, 111)),
        ("2", "w_2_1", (0, 51), (0, 111)),
    ],
    "2": [
        ("0a", "w_0_2", (0, 128), (0, 51)),
        ("0b", "w_0_2", (128, 228), (0, 51)),
        ("1", "w_1_2", (0, 111), (0, 51)),
    ],
}
# target chunk -> (bias keys summed, row slice)
BIASES = {
    "0a": (("b_1_0", "b_2_0"), (0, 128)),
    "0b": (("b_1_0", "b_2_0"), (128, 228)),
    "1": (("b_0_1", "b_2_1"), (0, 111)),
    "2": (("b_0_2", "b_1_2"), (0, 51)),
}
# column offsets of each lhsT piece inside the packed weight blob: every piece
# occupies a full 128-col slot (zero-padded) so LdWeights is FWL-eligible
WSLOT = {  # ordered by first use (TGT_ORDER_1) so early slots DMA first
    ("1", "0a"): 0, ("1", "0b"): 1, ("1", "2"): 2,
    ("2", "0a"): 3, ("2", "0b"): 4, ("2", "1"): 5,
    ("0a", "1"): 6, ("0a", "2"): 7, ("0b", "1"): 8, ("0b", "2"): 9,
}
WBLOB_COLS = 128 * len(WSLOT)  # 1280
TGT_ORDER_1 = ["1", "2", "0a", "0b"]   # iter1: produce deep-chain srcs first
TGT_ORDER_2 = ["0a", "0b", "1", "2"]   # iter2: consume earliest-ready srcs
TGT_COL = {"0a": 0, "0b": 1, "1": 2, "2": 3}

LAST_RESULTS = None  # stashed BassKernelResults for test harness introspection
SPLIT_LAST = True   # split the final macro tile in half to shrink the tail
SPLIT_FIRST = True   # split the first macro tile in half for faster pipeline fill
WARMUP_MMS = 32  # PE warmup matmul count (~3.1us at mid p-state)
TAIL_GPSIMD_SPAN = 1  # trailing MACRO-widths of out-DMA routed to the Pool
# SWDGE ring (SDMA engines 10-15, otherwise idle) so the final drain runs
# concurrently with the sync-ring backlog
DEDUP_LDW = True  # drop redundant consecutive same-weight InstLdweights so
# the PE's 64-deep reorder window can prefetch the next group's weights into
# the background weight buffer while the current group's matmuls stream


def _legalize_dedup_ldweights(ordered, nc):
    """Post-legalize pass: drop an InstLdweights whose weights AP is identical
    to the previous PE weight load with only matmuls in between -- the PE
    array still holds those weights, so the reload is pure overhead."""
    for bbname, insts in list(ordered.items()):
        new = []
        prev_key = None
        for inst in insts:
            if isinstance(inst, mybir.InstLdweights):
                si = inst.sync_info
                clean = not (si and (si.on_wait or si.on_update))
                key = repr(inst.ins[0])
                if clean and key == prev_key:
                    continue
                prev_key = key
            elif isinstance(inst, mybir.InstMatmult):
                pass  # matmuls leave the loaded weights intact
            elif getattr(inst, "engine", None) == mybir.EngineType.PE:
                prev_key = None  # other PE instruction: be conservative
            new.append(inst)
        ordered[bbname] = new
    return ordered


def _build_graph(alpha: float):
    if DEDUP_LDW:
        orig_legalize = tile.tile_legalize

        def legalize_and_dedup(ordered, nc_):
            return _legalize_dedup_ldweights(orig_legalize(ordered, nc_), nc_)

        tile.tile_legalize = legalize_and_dedup
        try:
            return _build_graph_inner(alpha)
        finally:
            tile.tile_legalize = orig_legalize
    return _build_graph_inner(alpha)


def _build_graph_inner(alpha: float):
    nc = bacc.Bacc("TRN2", target_bir_lowering=False, debug=False,
                   num_devices=NCORES)
    x_ext = nc.declare_dram_parameter("x", [512, PX], BF, isOutput=False)
    y_ext = nc.declare_dram_parameter("y", [CTOT, PX], BF, isOutput=True)
    w_ext = nc.declare_dram_parameter("wblob", [128, WBLOB_COLS], BF,
                                      isOutput=False)
    b_ext = nc.declare_dram_parameter("bblob", [128, 4], F32, isOutput=False)

    with tile.TileContext(nc) as tc:
        with (
            tc.tile_pool(name="wpool", bufs=1) as wpool,
            tc.tile_pool(name="xpool", bufs=16) as xpool,
            tc.tile_pool(name="hpool", bufs=12) as hpool,
            tc.tile_pool(name="opool", bufs=8) as opool,
            tc.tile_pool(name="mpool", bufs=6) as mpool,
            tc.tile_pool(name="spool", bufs=6) as spool,
            tc.tile_pool(name="pspool", bufs=4, space="PSUM") as pspool,
        ):
            wtile = wpool.tile([128, WBLOB_COLS], BF, tag="wblob")
            nc.sync.dma_start(wtile[0:128, 0:384], w_ext[0:128, 0:384])
            btile = wpool.tile([128, 4], F32, tag="bblob")
            nc.sync.dma_start(btile[:], b_ext[:])
            nc.sync.dma_start(wtile[0:128, 384:WBLOB_COLS],
                              w_ext[0:128, 384:WBLOB_COLS])
            wt = {}
            bt = {}
            for tgt, srcs in TARGETS.items():
                bt[tgt] = btile[0:128, TGT_COL[tgt]:TGT_COL[tgt] + 1]
                for (src, wkey, (k0, k1), _m) in srcs:
                    off = WSLOT[(tgt, src)] * 128
                    # K zero-padded to 128 (blob rows k1-k0..127 are zeros):
                    # FWL fast weight load requires K==128, and the padding
                    # rows multiply against defined-but-ignored rhs rows
                    wt[(tgt, src)] = wtile[0:128, off:off + 128]

            # PE warmup burst: dummy matmuls while the first input DMAs are in
            # flight, so the HAM clock-gate opens before the real matmul
            # stream starts.
            wu_w = wt[("1", "0a")]
            wu_ps = pspool.tile([128, MACRO // 2], F32, tag="ps")
            for _ in range(WARMUP_MMS):
                nc.tensor.matmul(wu_ps[0:128, 0:128], wu_w, wu_w,
                                 start=True, stop=True)

            def run_iter(src_tiles, out_pool, order, size):
                out = {}
                for tgt in order:
                    srcs = TARGETS[tgt]
                    rows = CHUNKS[tgt][1]
                    # psum split into halves: ACT drains half0 while half1's
                    # matmuls still stream; same weight-residency (halves are
                    # consecutive per source, so no extra LdWeights)
                    size2 = size // 2 if size >= 1024 else size
                    nh = size // size2
                    pts = []
                    for h in range(nh):
                        pt_h = pspool.tile([128, MACRO // 2], F32, tag="ps")
                        pts.append(pt_h)
                    for i, (src, wkey, (k0, k1), _m) in enumerate(srcs):
                        for h in range(nh):
                            for n in range(size2 // 512):
                                c0 = n * 512
                                nc.tensor.matmul(
                                    pts[h][0:128, c0:c0 + 512],
                                    wt[(tgt, src)],
                                    src_tiles[src][0:128,
                                                   h * size2 + c0:
                                                   h * size2 + c0 + 512],
                                    start=(i == 0),
                                    stop=(i == len(srcs) - 1),
                                )
                    # elementwise over all 128 rows (same cost: engines price
                    # by columns) so h padding rows stay defined zeros for the
                    # K=128 matmul reads in the next iteration
                    msg = mpool.tile([128, size], BF, tag="m")
                    for h in range(nh):
                        nc.scalar.activation(
                            msg[0:128, h * size2:(h + 1) * size2],
                            pts[h][0:128, 0:size2],
                            mybir.ActivationFunctionType.Prelu,
                            bias=bt[tgt], scale=1.0, alpha=alpha,
                        )
                    s = spool.tile([128, size], BF, tag="s")
                    nc.vector.tensor_add(s[0:128, :], src_tiles[tgt][0:128, :],
                                         msg[0:128, :])
                    h = out_pool.tile([128, size], BF,
                                      tag="h" if out_pool is hpool else "o")
                    nc.vector.tensor_scalar_max(h[0:128, :], s[0:128, :], 0.0)
                    out[tgt] = h
                return out

            # output rings split by chunk: 0a (128 rows, spreads over all 16
            # SDMA engines) + 2 ride the sync HWDGE ring; 0b + 1 (211 rows,
            # which would otherwise pile onto low engines) ride the Pool SWDGE
            # ring (engines 10-15)
            macro_cols = [(m * MACRO, MACRO) for m in range(NMACRO - 1)]
            if SPLIT_FIRST:
                macro_cols.pop(0)
                macro_cols = [(0, MACRO // 2), (MACRO // 2, MACRO // 2)] + macro_cols
            b0 = (NMACRO - 1) * MACRO
            macro_cols += [(b0, 1024), (b0 + 1024, 512), (b0 + 1536, 512)]
            XORDER = ["0a", "0b", "2", "1"]  # first targets need 0a/0b/2 first

            def load_macro(mcol, size):
                xs = {}
                for c in XORDER:
                    g0, rows, p0 = CHUNKS[c]
                    t = xpool.tile([128, size], BF, tag="x")
                    nc.sync.dma_start(t[:], x_ext[p0:p0 + 128, mcol:mcol + size])
                    xs[c] = t
                return xs

            # issue input DMA triggers LEAD macros ahead of compute so they
            # sit in front of the output triggers (which wait on compute) in
            # the in-order sync queue; otherwise input prefetch depth
            # collapses to ~1 macro and matmuls stall on x-chunk semaphores
            LEAD = 3
            loaded = [load_macro(*macro_cols[i])
                      for i in range(min(LEAD, len(macro_cols)))]

            def run_iter2_and_store(ph1, pmcol, psize):
                h2 = run_iter(ph1, opool, TGT_ORDER_2, psize)
                for c, (g0, rows, p0) in CHUNKS.items():
                    oeng = nc.sync if c in ("0a", "2") else nc.gpsimd
                    oeng.dma_start(y_ext[g0:g0 + rows, pmcol:pmcol + psize],
                                   h2[c][0:rows, :])

            # software-pipeline the two message-passing iterations one macro
            # apart: iter2(m-1) is emitted after iter1(m), so its h1 inputs
            # finished the ACT->add->relu chain a full macro earlier and the
            # PE never stalls on elementwise completions
            # depth 2 early/middle (h1 ages two macros before iter2 reads it,
            # giving the 80%-busy ACT engine slack), draining to depth 1 near
            # the end so the iter2-only tail stays short
            pending = []
            for m, (mcol, size) in enumerate(macro_cols):
                if m + LEAD < len(macro_cols):
                    loaded.append(load_macro(*macro_cols[m + LEAD]))
                h1 = run_iter(loaded[m], hpool, TGT_ORDER_1, size)
                pending.append((h1, mcol, size))
                depth = 2 if m < len(macro_cols) - 3 else 1
                while len(pending) > depth:
                    run_iter2_and_store(*pending.pop(0))
            for p in pending:
                run_iter2_and_store(*p)
    nc.compile()
    return nc


_GRAPH_CACHE = {}


def _get_graph(alpha: float):
    key = round(float(alpha), 8)
    if key not in _GRAPH_CACHE:
        _GRAPH_CACHE[key] = _build_graph(float(alpha))
    return _GRAPH_CACHE[key]


def _host_inputs(inputs):
    """Build per-core in_maps from full inputs."""
    xs = [np.asarray(inputs["x0"]), np.asarray(inputs["x1"]),
          np.asarray(inputs["x2"])]
    # weights / biases shared across cores, packed into single blobs
    wblob = np.zeros((128, WBLOB_COLS), dtype=BF16)
    bblob = np.zeros((128, 4), dtype=np.float32)
    for tgt, srcs in TARGETS.items():
        keys, (r0, r1) = BIASES[tgt]
        bsum = (np.asarray(inputs[keys[0]]) + np.asarray(inputs[keys[1]]))
        bblob[0:r1 - r0, TGT_COL[tgt]] = bsum[r0:r1].astype(np.float32)
        for (src, wkey, (k0, k1), (m0, m1)) in srcs:
            wT = np.asarray(inputs[wkey]).T  # [cin, cout]
            off = WSLOT[(tgt, src)] * 128
            wblob[0:k1 - k0, off:off + (m1 - m0)] = wT[k0:k1, m0:m1].astype(BF16)
    shared = {"wblob": wblob, "bblob": bblob}

    in_maps = []
    for k in range(NCORES):
        b = k // (NCORES // B)
        h0 = (k % (NCORES // B)) * ROWS_PER_CORE
        xp = np.zeros((512, PX), dtype=BF16)
        xp[0:128] = xs[0][b, 0:128, h0:h0 + ROWS_PER_CORE, :].reshape(128, PX)
        xp[128:228] = xs[0][b, 128:228, h0:h0 + ROWS_PER_CORE, :].reshape(100, PX)
        xp[256:367] = xs[1][b, :, h0:h0 + ROWS_PER_CORE, :].reshape(111, PX)
        xp[384:435] = xs[2][b, :, h0:h0 + ROWS_PER_CORE, :].reshape(51, PX)
        m = dict(shared)
        m["x"] = xp
        in_maps.append(m)
    return in_maps


def kernel(**inputs) -> np.ndarray:
    global LAST_RESULTS
    alpha = float(np.asarray(inputs["prelu_a"]).reshape(-1)[0])
    nc = _get_graph(alpha)
    in_maps = _host_inputs(inputs)
    trace = bool(os.environ.get("KERNEL_TRACE"))
    res = run_bass_kernel_spmd(nc, in_maps, list(range(NCORES)), trace=trace)
    LAST_RESULTS = res
    out = np.empty((B, CTOT, H, W), dtype=np.float32)
    for k in range(NCORES):
        b = k // (NCORES // B)
        h0 = (k % (NCORES // B)) * ROWS_PER_CORE
        y = np.asarray(res.results[k]["y"]).astype(np.float32)
        out[b, :, h0:h0 + ROWS_PER_CORE, :] = y.reshape(CTOT, ROWS_PER_CORE, W)
    return out



# revision 53
# speedup vs baseline: 1.0255x; 1.0255x over previous
"""CRFVGG_prune message-passing kernel for 8 TRN2 NeuronCores.

Structure: 3 node types with channel counts [228, 111, 51] hold [B,C,256,256]
feature maps. Two message-passing iterations of 1x1 convs (per-pixel matmuls)
between all ordered node pairs, each followed by prelu + residual + relu.

Sharding: pure data parallel over B*H rows (512 rows -> 64 rows/core).
Per-core layout is channels-on-partitions: x [390, 16384px] bf16 in four
chunks (h0a|h0b|h1|h2). All matmuls are bf16 (fp32 PSUM accum); every weight
tile is zero-padded to 128 columns so LdWeights takes the FWL fast path.
Elementwise in bf16, output written bf16 and upcast on host.
"""
import os
import sys

sys.path.insert(0, "/opt/trn_rl_repo")

import numpy as np
import ml_dtypes

import concourse.bass as bass
import concourse.tile as tile
from concourse import bacc, mybir
from concourse.bass_utils import run_bass_kernel_spmd

BF16 = ml_dtypes.bfloat16
BF = mybir.dt.bfloat16
F32 = mybir.dt.float32

B, H, W = 2, 256, 256
CHS = [228, 111, 51]
CTOT = sum(CHS)  # 390
NCORES = 8
ROWS_PER_CORE = (B * H) // NCORES  # 64
PX = ROWS_PER_CORE * W  # 16384 pixels per core
MACRO = 2048  # pixels per macro tile (one PSUM tile = 4 banks; 2 double-buffer)
NMACRO = PX // MACRO
NSUB = MACRO // 512  # matmul N-subtiles per macro

# chunk: (out_row_start in 390-layout, n_rows, x_row_start in padded 512-layout)
CHUNKS = {
    "0a": (0, 128, 0),
    "0b": (128, 100, 128),
    "1": (228, 111, 256),
    "2": (339, 51, 384),
}
# target chunk -> list of (source chunk, weight key, K-slice, M-slice)
# weight w_j_i: [cout_i, cin_j]; lhsT piece = w_j_i.T[kslice, mslice]
TARGETS = {
    "0a": [("1", "w_1_0", (0, 111), (0, 128)), ("2", "w_2_0", (0, 51), (0, 128))],
    "0b": [("1", "w_1_0", (0, 111), (128, 228)), ("2", "w_2_0", (0, 51), (128, 228))],
    "1": [
        ("0a", "w_0_1", (0, 128), (0, 111)),
        ("0b", "w_0_1", (128, 228), (0, 111)),
        ("2", "w_2_1", (0, 51), (0, 111)),
    ],
    "2": [
        ("0a", "w_0_2", (0, 128), (0, 51)),
        ("0b", "w_0_2", (128, 228), (0, 51)),
        ("1", "w_1_2", (0, 111), (0, 51)),
    ],
}
# target chunk -> (bias keys summed, row slice)
BIASES = {
    "0a": (("b_1_0", "b_2_0"), (0, 128)),
    "0b": (("b_1_0", "b_2_0"), (128, 228)),
    "1": (("b_0_1", "b_2_1"), (0, 111)),
    "2": (("b_0_2", "b_1_2"), (0, 51)),
}
# column offsets of each lhsT piece inside the packed weight blob: every piece
# occupies a full 128-col slot (zero-padded) so LdWeights is FWL-eligible
WSLOT = {  # ordered by first use (TGT_ORDER_1) so early slots DMA first
    ("1", "0a"): 0, ("1", "0b"): 1, ("1", "2"): 2,
    ("2", "0a"): 3, ("2", "0b"): 4, ("2", "1"): 5,
    ("0a", "1"): 6, ("0a", "2"): 7, ("0b", "1"): 8, ("0b", "2"): 9,
}
WBLOB_COLS = 128 * len(WSLOT)  # 1280
TGT_ORDER_1 = ["1", "2", "0a", "0b"]   # iter1: produce deep-chain srcs first
TGT_ORDER_2 = ["0a", "0b", "1", "2"]   # iter2: consume earliest-ready srcs
TGT_COL = {"0a": 0, "0b": 1, "1": 2, "2": 3}

LAST_RESULTS = None  # stashed BassKernelResults for test harness introspection
SPLIT_LAST = True   # split the final macro tile in half to shrink the tail
SPLIT_FIRST = False  # obsolete: LEAD-3 input prefetch already fills the pipe,
# and the 1024/2048 size transition caused elementwise-backlog stalls
WARMUP_MMS = 32  # PE warmup matmul count (~3.1us at mid p-state)
TAIL_GPSIMD_SPAN = 1  # trailing MACRO-widths of out-DMA routed to the Pool
# SWDGE ring (SDMA engines 10-15, otherwise idle) so the final drain runs
# concurrently with the sync-ring backlog
DEDUP_LDW = True  # drop redundant consecutive same-weight InstLdweights so
# the PE's 64-deep reorder window can prefetch the next group's weights into
# the background weight buffer while the current group's matmuls stream


def _legalize_dedup_ldweights(ordered, nc):
    """Post-legalize pass: drop an InstLdweights whose weights AP is identical
    to the previous PE weight load with only matmuls in between -- the PE
    array still holds those weights, so the reload is pure overhead."""
    for bbname, insts in list(ordered.items()):
        new = []
        prev_key = None
        for inst in insts:
            if isinstance(inst, mybir.InstLdweights):
                si = inst.sync_info
                clean = not (si and (si.on_wait or si.on_update))
                key = repr(inst.ins[0])
                if clean and key == prev_key:
                    continue
                prev_key = key
            elif isinstance(inst, mybir.InstMatmult):
                pass  # matmuls leave the loaded weights intact
            elif getattr(inst, "engine", None) == mybir.EngineType.PE:
                prev_key = None  # other PE instruction: be conservative
            new.append(inst)
        ordered[bbname] = new
    return ordered


def _build_graph(alpha: float):
    if DEDUP_LDW:
        orig_legalize = tile.tile_legalize

        def legalize_and_dedup(ordered, nc_):
            return _legalize_dedup_ldweights(orig_legalize(ordered, nc_), nc_)

        tile.tile_legalize = legalize_and_dedup
        try:
            return _build_graph_inner(alpha)
        finally:
            tile.tile_legalize = orig_legalize
    return _build_graph_inner(alpha)


def _build_graph_inner(alpha: float):
    nc = bacc.Bacc("TRN2", target_bir_lowering=False, debug=False,
                   num_devices=NCORES)
    x_ext = nc.declare_dram_parameter("x", [512, PX], BF, isOutput=False)
    y_ext = nc.declare_dram_parameter("y", [CTOT, PX], BF, isOutput=True)
    w_ext = nc.declare_dram_parameter("wblob", [128, WBLOB_COLS], BF,
                                      isOutput=False)
    b_ext = nc.declare_dram_parameter("bblob", [128, 4], F32, isOutput=False)

    with tile.TileContext(nc) as tc:
        with (
            tc.tile_pool(name="wpool", bufs=1) as wpool,
            tc.tile_pool(name="xpool", bufs=16) as xpool,
            tc.tile_pool(name="hpool", bufs=8) as hpool,
            tc.tile_pool(name="opool", bufs=8) as opool,
            tc.tile_pool(name="mpool", bufs=6) as mpool,
            tc.tile_pool(name="spool", bufs=6) as spool,
            tc.tile_pool(name="pspool", bufs=4, space="PSUM") as pspool,
        ):
            wtile = wpool.tile([128, WBLOB_COLS], BF, tag="wblob")
            nc.sync.dma_start(wtile[0:128, 0:384], w_ext[0:128, 0:384])
            btile = wpool.tile([128, 4], F32, tag="bblob")
            nc.sync.dma_start(btile[:], b_ext[:])
            nc.sync.dma_start(wtile[0:128, 384:WBLOB_COLS],
                              w_ext[0:128, 384:WBLOB_COLS])
            wt = {}
            bt = {}
            for tgt, srcs in TARGETS.items():
                bt[tgt] = btile[0:128, TGT_COL[tgt]:TGT_COL[tgt] + 1]
                for (src, wkey, (k0, k1), _m) in srcs:
                    off = WSLOT[(tgt, src)] * 128
                    # K zero-padded to 128 (blob rows k1-k0..127 are zeros):
                    # FWL fast weight load requires K==128, and the padding
                    # rows multiply against defined-but-ignored rhs rows
                    wt[(tgt, src)] = wtile[0:128, off:off + 128]

            # PE warmup burst: dummy matmuls while the first input DMAs are in
            # flight, so the HAM clock-gate opens before the real matmul
            # stream starts.
            wu_w = wt[("1", "0a")]
            wu_ps = pspool.tile([128, MACRO // 2], F32, tag="ps")
            for _ in range(WARMUP_MMS):
                nc.tensor.matmul(wu_ps[0:128, 0:128], wu_w, wu_w,
                                 start=True, stop=True)

            def run_iter(src_tiles, out_pool, order, size):
                out = {}
                for tgt in order:
                    srcs = TARGETS[tgt]
                    rows = CHUNKS[tgt][1]
                    # psum split into halves: ACT drains half0 while half1's
                    # matmuls still stream; same weight-residency (halves are
                    # consecutive per source, so no extra LdWeights)
                    size2 = size // 2 if size >= 1024 else size
                    nh = size // size2
                    pts = []
                    for h in range(nh):
                        pt_h = pspool.tile([128, MACRO // 2], F32, tag="ps")
                        pts.append(pt_h)
                    for i, (src, wkey, (k0, k1), _m) in enumerate(srcs):
                        for h in range(nh):
                            for n in range(size2 // 512):
                                c0 = n * 512
                                nc.tensor.matmul(
                                    pts[h][0:128, c0:c0 + 512],
                                    wt[(tgt, src)],
                                    src_tiles[src][0:128,
                                                   h * size2 + c0:
                                                   h * size2 + c0 + 512],
                                    start=(i == 0),
                                    stop=(i == len(srcs) - 1),
                                )
                    # elementwise over all 128 rows (same cost: engines price
                    # by columns) so h padding rows stay defined zeros for the
                    # K=128 matmul reads in the next iteration
                    msg = mpool.tile([128, size], BF, tag="m")
                    for h in range(nh):
                        nc.scalar.activation(
                            msg[0:128, h * size2:(h + 1) * size2],
                            pts[h][0:128, 0:size2],
                            mybir.ActivationFunctionType.Prelu,
                            bias=bt[tgt], scale=1.0, alpha=alpha,
                        )
                    s = spool.tile([128, size], BF, tag="s")
                    nc.vector.tensor_add(s[0:128, :], src_tiles[tgt][0:128, :],
                                         msg[0:128, :])
                    h = out_pool.tile([128, size], BF,
                                      tag="h" if out_pool is hpool else "o")
                    nc.vector.tensor_scalar_max(h[0:128, :], s[0:128, :], 0.0)
                    out[tgt] = h
                return out

            # output rings split by chunk: 0a (128 rows, spreads over all 16
            # SDMA engines) + 2 ride the sync HWDGE ring; 0b + 1 (211 rows,
            # which would otherwise pile onto low engines) ride the Pool SWDGE
            # ring (engines 10-15)
            macro_cols = [(m * MACRO, MACRO) for m in range(NMACRO - 1)]
            if SPLIT_FIRST:
                macro_cols.pop(0)
                macro_cols = [(0, MACRO // 2), (MACRO // 2, MACRO // 2)] + macro_cols
            b0 = (NMACRO - 1) * MACRO
            macro_cols += [(b0, 1024), (b0 + 1024, 512), (b0 + 1536, 512)]
            XORDER = ["0a", "0b", "2", "1"]  # first targets need 0a/0b/2 first

            def load_macro(mcol, size):
                xs = {}
                for c in XORDER:
                    g0, rows, p0 = CHUNKS[c]
                    t = xpool.tile([128, size], BF, tag="x")
                    nc.sync.dma_start(t[:], x_ext[p0:p0 + 128, mcol:mcol + size])
                    xs[c] = t
                return xs

            # issue input DMA triggers LEAD macros ahead of compute so they
            # sit in front of the output triggers (which wait on compute) in
            # the in-order sync queue; otherwise input prefetch depth
            # collapses to ~1 macro and matmuls stall on x-chunk semaphores
            LEAD = 3
            loaded = [load_macro(*macro_cols[i])
                      for i in range(min(LEAD, len(macro_cols)))]

            def run_iter2_and_store(ph1, pmcol, psize):
                h2 = run_iter(ph1, opool, TGT_ORDER_2, psize)
                for c, (g0, rows, p0) in CHUNKS.items():
                    oeng = nc.sync if c in ("0a", "2") else nc.gpsimd
                    oeng.dma_start(y_ext[g0:g0 + rows, pmcol:pmcol + psize],
                                   h2[c][0:rows, :])

            # software-pipeline the two message-passing iterations one macro
            # apart: iter2(m-1) is emitted after iter1(m), so its h1 inputs
            # finished the ACT->add->relu chain a full macro earlier and the
            # PE never stalls on elementwise completions
            pending = None
            for m, (mcol, size) in enumerate(macro_cols):
                if m + LEAD < len(macro_cols):
                    loaded.append(load_macro(*macro_cols[m + LEAD]))
                h1 = run_iter(loaded[m], hpool, TGT_ORDER_1, size)
                if pending is not None:
                    run_iter2_and_store(*pending)
                pending = (h1, mcol, size)
            run_iter2_and_store(*pending)
    nc.compile()
    return nc


_GRAPH_CACHE = {}


def _get_graph(alpha: float):
    key = round(float(alpha), 8)
    if key not in _GRAPH_CACHE:
        _GRAPH_CACHE[key] = _build_graph(float(alpha))
    return _GRAPH_CACHE[key]


def _host_inputs(inputs):
    """Build per-core in_maps from full inputs."""
    xs = [np.asarray(inputs["x0"]), np.asarray(inputs["x1"]),
          np.asarray(inputs["x2"])]
    # weights / biases shared across cores, packed into single blobs
    wblob = np.zeros((128, WBLOB_COLS), dtype=BF16)
    bblob = np.zeros((128, 4), dtype=np.float32)
    for tgt, srcs in TARGETS.items():
        keys, (r0, r1) = BIASES[tgt]
        bsum = (np.asarray(inputs[keys[0]]) + np.asarray(inputs[keys[1]]))
        bblob[0:r1 - r0, TGT_COL[tgt]] = bsum[r0:r1].astype(np.float32)
        for (src, wkey, (k0, k1), (m0, m1)) in srcs:
            wT = np.asarray(inputs[wkey]).T  # [cin, cout]
            off = WSLOT[(tgt, src)] * 128
            wblob[0:k1 - k0, off:off + (m1 - m0)] = wT[k0:k1, m0:m1].astype(BF16)
    shared = {"wblob": wblob, "bblob": bblob}

    in_maps = []
    for k in range(NCORES):
        b = k // (NCORES // B)
        h0 = (k % (NCORES // B)) * ROWS_PER_CORE
        xp = np.zeros((512, PX), dtype=BF16)
        xp[0:128] = xs[0][b, 0:128, h0:h0 + ROWS_PER_CORE, :].reshape(128, PX)
        xp[128:228] = xs[0][b, 128:228, h0:h0 + ROWS_PER_CORE, :].reshape(100, PX)
        xp[256:367] = xs[1][b, :, h0:h0 + ROWS_PER_CORE, :].reshape(111, PX)
        xp[384:435] = xs[2][b, :, h0:h0 + ROWS_PER_CORE, :].reshape(51, PX)
        m = dict(shared)
        m["x"] = xp
        in_maps.append(m)
    return in_maps


def kernel(**inputs) -> np.ndarray:
    global LAST_RESULTS
    alpha = float(np.asarray(inputs["prelu_a"]).reshape(-1)[0])
    nc = _get_graph(alpha)
    in_maps = _host_inputs(inputs)
    trace = bool(os.environ.get("KERNEL_TRACE"))
    res = run_bass_kernel_spmd(nc, in_maps, list(range(NCORES)), trace=trace)
    LAST_RESULTS = res
    out = np.empty((B, CTOT, H, W), dtype=np.float32)
    for k in range(NCORES):
        b = k // (NCORES // B)
        h0 = (k % (NCORES // B)) * ROWS_PER_CORE
        y = np.asarray(res.results[k]["y"]).astype(np.float32)
        out[b, :, h0:h0 + ROWS_PER_CORE, :] = y.reshape(CTOT, ROWS_PER_CORE, W)
    return out

